# revision 1
# baseline (speedup 1.0000x reference)
"""MultiHeadAttention Trainium2 kernel (8 NeuronCores, SPMD).

Reference computation (B=4, T=1024, D=768, H=12, Dh=64):
    q = x @ Wq.T ; k = x @ Wk.T ; v = x @ Wv.T       (per-head reshape)
    attn = softmax((q @ k.T) / 8)
    out = (attn @ v) @ Wo.T + bo

Sharding: 8 cores = 4 batches x 2 head-halves (6 heads each). Each core
computes a [1024, 768] partial of the output projection for its 6 heads;
the host sums the two partials per batch and adds the bias.

Per-core dataflow (all matmuls fp32 data, fp32r PE mode):
    xT [768,1024] (host-pretransposed) -> SBUF
    qT,kT = (W x)  in [384,1024] layout; v in [1024,384] layout
    S.T tiles [kt=128, qt=512] = kT_head.T @ qT_head   (K=64 contraction)
    expS = exp(S.T) via ScalarE reading PSUM
    ctx.T psum [65, qt] = [v_head | ones_col].T @ expS  (K=kt accumulate)
        rows 0:64 = unnormalized ctx.T, row 64 = softmax denominator
        (the ones column makes the denominator free in the same stream)
    ctxT_norm = ctx.T * 1/denom   (DVE recip + gpsimd partition_broadcast)
    out_partial = ctxT_norm.T @ Wo_slice.T             (K=384 accumulate)
"""

import numpy as np

import concourse.mybir as mybir
from concourse import bacc
from concourse.tile import TileContext
from concourse.bass_utils import run_bass_kernel_spmd

FP = mybir.dt.float32
FPR = mybir.dt.float32r
AF = mybir.ActivationFunctionType

B, T, D = 4, 1024, 768
H, DH = 12, 64
NCORES = 8
HPC = 6           # heads per core
DPC = HPC * DH    # 384 head-dims per core
KC = D // 128     # 6 contraction chunks for d_in
MC = DPC // 128   # 3 chunks of per-core head dims
NT = T // 512     # 2 free-dim tiles of tokens
TT = T // 128     # 8 partition tiles of tokens


def emit_mha(tc, xT, wq, wk, wv, wo, ones, out, ctx):
    nc = tc.nc

    singles = ctx.enter_context(tc.tile_pool(name="singles", bufs=1))
    proj_psum = ctx.enter_context(tc.tile_pool(name="proj_psum", bufs=2, space="PSUM"))
    scores_psum = ctx.enter_context(
        tc.tile_pool(name="scores_psum", bufs=2, space="PSUM")
    )
    ctx_psum = ctx.enter_context(tc.tile_pool(name="ctx_psum", bufs=2, space="PSUM"))
    expS_pool = ctx.enter_context(tc.tile_pool(name="expS", bufs=10))
    rcp_pool = ctx.enter_context(tc.tile_pool(name="rcp", bufs=10))
    out_pool = ctx.enter_context(tc.tile_pool(name="outsb", bufs=6))

    # ---------------- staged input DMAs ----------------
    # xT first (QKV-critical), chunked so matmul accumulation chases arrival;
    # wq/wk sliced per m-tile so head-pair hp unblocks after slice hp.
    xT_sb = singles.tile([128, KC, T], FPR, name="xT_sb", tag="xT_sb")
    xTr = xT.rearrange("(c p) t -> p c t", p=128).bitcast(FPR)
    wq_sb = singles.tile([128, KC, DPC], FPR, name="wq_sb", tag="wq_sb")
    wk_sb = singles.tile([128, KC, DPC], FPR, name="wk_sb", tag="wk_sb")
    wv_sb = singles.tile([128, KC, DPC], FPR, name="wv_sb", tag="wv_sb")
    wo_sb = singles.tile([128, MC, D], FPR, name="wo_sb", tag="wo_sb")
    wqr = wq.rearrange("(c p) d -> p c d", p=128).bitcast(FPR)
    wkr = wk.rearrange("(c p) d -> p c d", p=128).bitcast(FPR)
    # first q/k slice weights, then xT token-halves (n0 of every chunk first
    # so the first scores tile unblocks after half the xT bytes)
    for c in range(KC):
        nc.sync.dma_start(out=xT_sb[:, c, 0:512], in_=xTr[:, c, 0:512])
        if c == 1:
            nc.sync.dma_start(out=wk_sb[:, :, 0:128], in_=wkr[:, :, 0:128])
            nc.sync.dma_start(out=wq_sb[:, :, 0:128], in_=wqr[:, :, 0:128])
    for c in range(KC):
        nc.sync.dma_start(out=xT_sb[:, c, 512:1024], in_=xTr[:, c, 512:1024])
    nc.sync.dma_start(out=wv_sb, in_=wv.rearrange("(c p) d -> p c d", p=128).bitcast(FPR))
    for m in range(1, MC):
        nc.sync.dma_start(out=wk_sb[:, :, m * 128 : (m + 1) * 128],
                          in_=wkr[:, :, m * 128 : (m + 1) * 128])
        nc.sync.dma_start(out=wq_sb[:, :, m * 128 : (m + 1) * 128],
                          in_=wqr[:, :, m * 128 : (m + 1) * 128])
    nc.sync.dma_start(out=wo_sb, in_=wo.rearrange("(c p) d -> p c d", p=128).bitcast(FPR))

    qT_sb = singles.tile([128, MC, T], FPR, name="qT_sb", tag="qT_sb")
    kT_sb = singles.tile([128, MC, T], FPR, name="kT_sb", tag="kT_sb")
    ctxT_sb = singles.tile([128, MC, T], FPR, name="ctxT_sb", tag="ctxT_sb")

    # v tiles [t_tile, 6 heads x (64 v cols + 1 ones col)]: the ones column
    # makes each head's ctx matmul also produce its softmax denominator
    # (psum row 64) in the same stream. memset can't write fp32r, so the
    # ones come from a tiny DRAM input.
    v_sb = []
    for i in range(TT):
        vt = singles.tile([128, HPC, DH + 1], FPR, name=f"v_sb{i}", tag=f"v_sb{i}")
        nc.sync.dma_start(out=vt[:, :, DH : DH + 1], in_=ones.bitcast(FPR))
        v_sb.append(vt)

    def qk_proj(m):
        # qT/kT chunk m: out[m=dout(128), n=t(512)] = sum_c w[c,m].T @ xT[c,n]
        for n in range(NT):
            for w_sb, dst in ((wk_sb, kT_sb), (wq_sb, qT_sb)):
                ps = proj_psum.tile([128, 512], FP, name="ps_qk", tag="proj")
                for c in range(KC):
                    nc.tensor.matmul(
                        ps,
                        lhsT=w_sb[:, c, m * 128 : (m + 1) * 128],
                        rhs=xT_sb[:, c, n * 512 : (n + 1) * 512],
                        start=(c == 0),
                        stop=(c == KC - 1),
                    )
                nc.vector.tensor_copy(dst[:, m, n * 512 : (n + 1) * 512], ps)

    def v_proj():
        # v: out[m=t_tile(128), n=dh(384)] = sum_c xT[c,m].T @ wv[c,n]
        for mt in range(TT):
            ps = proj_psum.tile([128, DPC], FP, name="ps_v", tag="proj")
            for c in range(KC):
                nc.tensor.matmul(
                    ps,
                    lhsT=xT_sb[:, c, mt * 128 : (mt + 1) * 128],
                    rhs=wv_sb[:, c, :],
                    start=(c == 0),
                    stop=(c == KC - 1),
                )
            nc.vector.tensor_copy(v_sb[mt][:, :, 0:DH], ps)

    def pair_scores(hp, qt):
        # heads 2hp (partitions 0:64) and 2hp+1 (partitions 64:128) of chunk
        # hp. Scores for the two heads are interleaved per-matmul: disjoint
        # PE row groups (tile_position rows 0 vs 64) can run concurrently.
        pair = []
        for h in (2 * hp, 2 * hp + 1):
            po = 64 * (h % 2)
            kT_h = kT_sb[po : po + 64, hp, :]
            qT_h = qT_sb[po : po + 64, hp, qt * 512 : (qt + 1) * 512]
            pair.append((h, po, kT_h, qT_h, []))

        for g in range(4):
            pss = [
                scores_psum.tile([128, 1024], FP, name="ps_s", tag="scores")
                for _ in pair
            ]
            for r2 in range(2):
                j = 2 * g + r2
                for (h, po, kT_h, qT_h, exps), ps in zip(pair, pss):
                    nc.tensor.matmul(
                        ps[:, r2 * 512 : (r2 + 1) * 512],
                        lhsT=kT_h[:, j * 128 : (j + 1) * 128],
                        rhs=qT_h,
                        start=True,
                        stop=True,
                    )
            for (h, po, kT_h, qT_h, exps), ps in zip(pair, pss):
                ex = expS_pool.tile([128, 1024], FPR, name="ex", tag="expS")
                nc.scalar.activation(ex, ps, AF.Exp)
                exps.append(ex)
        return pair

    def pair_ctx(hp, qt, pair, fine=False):
        for h, po, kT_h, qT_h, exps in pair:
            # ctx.T accumulate over kt: lhsT = [v_head | ones] (M=65):
            # psum rows 0:64 = unnormalized ctx.T, row 64 = denominator
            pc = ctx_psum.tile([128, 512], FP, name="pc", tag="ctx")
            for j in range(TT):
                ex_j = exps[j // 2][:, (j % 2) * 512 : (j % 2 + 1) * 512]
                nc.tensor.matmul(
                    pc[0:65, :],
                    lhsT=v_sb[j][:, h, :],
                    rhs=ex_j,
                    start=(j == 0),
                    stop=(j == TT - 1),
                )
            # 1/denom to partition 0, broadcast over 64 partitions
            # (gpsimd custom ops require base-partition-0 operands);
            # fine=True halves the chain for the schedule's tail
            rcp = rcp_pool.tile([1, 512], FP, name="rcp", tag="rcp")
            rcpb = rcp_pool.tile([64, 512], FP, name="rcpb", tag="rcpb")
            pieces = 2 if fine else 1
            w = 512 // pieces
            for p2 in range(pieces):
                cs = slice(p2 * w, (p2 + 1) * w)
                nc.vector.reciprocal(rcp[:, cs], pc[64:65, cs])
                nc.gpsimd.partition_broadcast(rcpb[:, cs], rcp[:, cs], channels=64)
                nc.vector.tensor_mul(
                    ctxT_sb[po : po + 64, hp, qt * 512 + p2 * w : qt * 512 + (p2 + 1) * w],
                    pc[0:64, cs],
                    rcpb[:, cs],
                )

    def out_proj(mts, split_dma=False):
        # out[m=t_tile(128), n=dout(384)] = sum_c ctxT[c,m].T @ wo[c,n];
        # the two psum->sbuf copies split across DVE and ScalarE (ACT is
        # idle by this phase) so the final DMAs unblock sooner.
        for mt in mts:
            osb = out_pool.tile([128, D], FP, name="osb", tag="outsb")
            for n2 in range(2):
                ps = proj_psum.tile([128, 384], FP, name="ps_o", tag="proj")
                for c in range(MC):
                    nc.tensor.matmul(
                        ps,
                        lhsT=ctxT_sb[:, c, mt * 128 : (mt + 1) * 128],
                        rhs=wo_sb[:, c, n2 * 384 : (n2 + 1) * 384],
                        start=(c == 0),
                        stop=(c == MC - 1),
                    )
                if n2 == 0:
                    nc.vector.tensor_copy(osb[:, 0:384], ps)
                else:
                    nc.scalar.copy(osb[:, 384:768], ps)
                if split_dma:
                    nc.sync.dma_start(
                        out=out[mt * 128 : (mt + 1) * 128, n2 * 384 : (n2 + 1) * 384],
                        in_=osb[:, n2 * 384 : (n2 + 1) * 384],
                    )
            if not split_dma:
                nc.sync.dma_start(out=out[mt * 128 : (mt + 1) * 128, :], in_=osb)

    # interleaved schedule: scores of head pair 0 start as soon as q/k chunk
    # 0 lands (ACT warms up early); v projection overlaps those exps; the
    # output projection's t-halves chase the last head pair's two qt halves.
    qk_proj(0)
    p00 = pair_scores(0, 0)
    v_proj()
    pair_ctx(0, 0, p00)
    p01 = pair_scores(0, 1)
    qk_proj(1)
    pair_ctx(0, 1, p01)
    p10 = pair_scores(1, 0)
    pair_ctx(1, 0, p10)
    p11 = pair_scores(1, 1)
    qk_proj(2)
    pair_ctx(1, 1, p11)
    p20 = pair_scores(2, 0)
    pair_ctx(2, 0, p20)
    p21 = pair_scores(2, 1)
    out_proj(range(0, TT // 2))
    pair_ctx(2, 1, p21, fine=True)
    out_proj(range(TT // 2, TT), split_dma=True)


_PROGRAM = None


def build_program():
    global _PROGRAM
    if _PROGRAM is not None:
        return _PROGRAM
    nc = bacc.Bacc("TRN2", target_bir_lowering=False, debug=False, num_devices=NCORES)
    xT = nc.dram_tensor("xT", (D, T), FP, kind="ExternalInput").ap()
    wq = nc.dram_tensor("wq", (D, DPC), FP, kind="ExternalInput").ap()
    wk = nc.dram_tensor("wk", (D, DPC), FP, kind="ExternalInput").ap()
    wv = nc.dram_tensor("wv", (D, DPC), FP, kind="ExternalInput").ap()
    wo = nc.dram_tensor("wo", (DPC, D), FP, kind="ExternalInput").ap()
    ones = nc.dram_tensor("ones", (128, HPC), FP, kind="ExternalInput").ap()
    out = nc.dram_tensor("out", (T, D), FP, kind="ExternalOutput").ap()
    from contextlib import ExitStack

    with TileContext(nc) as tc, ExitStack() as st:
        emit_mha(tc, xT, wq, wk, wv, wo, ones, out, st)
    nc.compile()
    _PROGRAM = nc
    return nc


def make_in_maps(x, Wq, Wk, Wv, Wo):
    x = np.asarray(x, dtype=np.float32)
    in_maps = []
    xTs = [np.ascontiguousarray(x[b].T) for b in range(B)]
    for core in range(NCORES):
        b, hh = core // 2, core % 2
        sl = slice(hh * DPC, (hh + 1) * DPC)
        in_maps.append(
            {
                "xT": xTs[b],
                "wq": np.ascontiguousarray((np.asarray(Wq)[sl] * 0.125).T, np.float32),
                "wk": np.ascontiguousarray(np.asarray(Wk)[sl].T, np.float32),
                "wv": np.ascontiguousarray(np.asarray(Wv)[sl].T, np.float32),
                "wo": np.ascontiguousarray(np.asarray(Wo)[:, sl].T, np.float32),
                "ones": np.ones((128, HPC), np.float32),
            }
        )
    return in_maps


def kernel(x, Wq, Wk, Wv, Wo, bo):
    nc = build_program()
    in_maps = make_in_maps(x, Wq, Wk, Wv, Wo)
    res = run_bass_kernel_spmd(nc, in_maps, core_ids=list(range(NCORES)))
    bo = np.asarray(bo, dtype=np.float32)
    out = np.empty((B, T, D), dtype=np.float32)
    for b in range(B):
        out[b] = res.results[2 * b]["out"] + res.results[2 * b + 1]["out"] + bo
    return out



# revision 2
# speedup vs baseline: 1.0901x; 1.0901x over previous
"""MultiHeadAttention Trainium2 kernel (8 NeuronCores, SPMD).

Reference computation (B=4, T=1024, D=768, H=12, Dh=64):
    q = x @ Wq.T ; k = x @ Wk.T ; v = x @ Wv.T       (per-head reshape)
    attn = softmax((q @ k.T) / 8)
    out = (attn @ v) @ Wo.T + bo

Sharding: 8 cores = 4 batches x 2 head-halves (6 heads each). Each core
computes a [1024, 768] partial of the output projection for its 6 heads;
the host sums the two partials per batch and adds the bias.

All device data is bf16 (host-converted, fp32 PSUM accumulation), which
runs matmuls at 1 cycle/row for any moving-dim size and halves DMA bytes.

Per-core dataflow:
    qT,kT = (W x)  in [dh(384), t] layout (per m-chunk of 128 = 2 heads)
    v     = (x Wv) in [t, 6*(64+1)] tiles; col 64 of each head block is a
            host of ones so the ctx matmul also emits softmax denominators
    S.T tiles [kt=128, q] = kT_head.T @ qT_head   (K=64 contraction)
    expS  = exp(S.T) bf16 via ScalarE reading PSUM
    ctx[q, 65] psum += expS_j.T @ [v_j | 1]       (K=kt chunks, N=65)
        col 64 = denominator; normalize with DVE reciprocal +
        per-partition tensor_scalar_mul (denominator is per-q = per-row)
    ctxT = PE-transpose(ctx_norm) per 2-head pair  (bf16, 128 cyc each)
    out[q, :] = sum_m ctxT_m.T @ Wo_m   split as psum(m0+m1) -> sbuf,
        then a tail-only m2 matmul + DVE add, so only the last head pair
        sits on the critical tail.
"""

import numpy as np
import ml_dtypes

import concourse.mybir as mybir
from concourse import bacc
from concourse.tile import TileContext
from concourse.bass_utils import run_bass_kernel_spmd

FP = mybir.dt.float32
BF = mybir.dt.bfloat16
AF = mybir.ActivationFunctionType
BF_NP = ml_dtypes.bfloat16

B, T, D = 4, 1024, 768
H, DH = 12, 64
NCORES = 8
HPC = 6           # heads per core
DPC = HPC * DH    # 384 head-dims per core
KC = D // 128     # 6 contraction chunks of d_in
MC = DPC // 128   # 3 chunks of per-core head dims (2 heads each)
QC = T // 128     # 8 query chunks
TT = T // 128     # 8 key chunks


def emit_mha(tc, xt, wk, wq, wv, wo, ident, out, ctx):
    nc = tc.nc

    singles = ctx.enter_context(tc.tile_pool(name="singles", bufs=1))
    # scores psum: [128,1024] fp32 = 2 banks each; 2 bufs = 4 banks
    sps = ctx.enter_context(tc.tile_pool(name="sps", bufs=2, space="PSUM"))
    # shared work psum (qk/v/ctx/transpose/out): 4 bufs x 1 bank = 4 banks
    wps = ctx.enter_context(tc.tile_pool(name="wps", bufs=4, space="PSUM"))
    expp = ctx.enter_context(tc.tile_pool(name="expp", bufs=34))
    osbp = ctx.enter_context(tc.tile_pool(name="osbp", bufs=8))

    # ---------------- SBUF singles ----------------
    xT_sb = singles.tile([128, KC, T], BF, name="xT_sb", tag="xT_sb")
    wk_sb = singles.tile([128, MC, 768], BF, name="wk_sb", tag="wk_sb")
    wq_sb = singles.tile([128, MC, 768], BF, name="wq_sb", tag="wq_sb")
    wv_sb = singles.tile([128, KC, DPC], BF, name="wv_sb", tag="wv_sb")
    wo_sb = singles.tile([128, MC, 768], BF, name="wo_sb", tag="wo_sb")
    id_sb = singles.tile([128, 128], BF, name="id_sb", tag="id_sb")
    kT_sb = singles.tile([128, MC, T], BF, name="kT_sb", tag="kT_sb")
    qT_sb = singles.tile([128, MC, T], BF, name="qT_sb", tag="qT_sb")
    ctxn_sb = singles.tile([128, QC, DPC], BF, name="ctxn_sb", tag="ctxn_sb")
    ctxT_sb = singles.tile([128, MC, T], BF, name="ctxT_sb", tag="ctxT_sb")
    rcp_sb = singles.tile([128, H * QC], FP, name="rcp_sb", tag="rcp_sb")
    v_sb = []
    for j in range(TT):
        vt = singles.tile([128, HPC, DH + 1], BF, name=f"v_sb{j}", tag=f"v_sb{j}")
        v_sb.append(vt)

    # ones columns for the fused softmax denominators (Pool engine, SBUF)
    for j in range(TT):
        nc.gpsimd.memset(v_sb[j][:, :, DH : DH + 1], 1.0)

    # ---------------- input DMAs (SP/HWDGE queue) ----------------
    # order = consumption order: first m-block weights, then x token-halves
    # (n0 of every chunk first so the qk chase starts after ~1/2 of x).
    xtr = xt.rearrange("p (c t) -> p c t", c=KC)
    nc.sync.dma_start(out=wk_sb[:, 0, :], in_=wk[:, 0:768])
    nc.sync.dma_start(out=wq_sb[:, 0, :], in_=wq[:, 0:768])
    for c in range(KC):
        nc.sync.dma_start(out=xT_sb[:, c, 0:512], in_=xtr[:, c, 0:512])
    for c in range(KC):
        nc.sync.dma_start(out=xT_sb[:, c, 512:1024], in_=xtr[:, c, 512:1024])
    nc.sync.dma_start(out=wk_sb[:, 1:3, :], in_=wk[:, 768:2304])
    nc.sync.dma_start(out=wq_sb[:, 1:3, :], in_=wq[:, 768:2304])
    nc.sync.dma_start(out=wv_sb, in_=wv.rearrange("p (c n) -> p c n", c=KC))
    nc.sync.dma_start(out=wo_sb, in_=wo.rearrange("p (m d) -> p m d", m=MC))
    nc.sync.dma_start(out=id_sb, in_=ident)

    # ---------------- atoms ----------------
    expS = {}

    def qk_half(m, n, w_sb, dst, act_copy):
        ps = wps.tile([128, 512], FP, name="ps_qk", tag="w")
        for c in range(KC):
            nc.tensor.matmul(
                ps,
                lhsT=w_sb[:, m, c * 128 : (c + 1) * 128],
                rhs=xT_sb[:, c, n * 512 : (n + 1) * 512],
                start=(c == 0),
                stop=(c == KC - 1),
            )
        if act_copy:
            nc.scalar.copy(dst[:, m, n * 512 : (n + 1) * 512], ps)
        else:
            nc.vector.tensor_copy(dst[:, m, n * 512 : (n + 1) * 512], ps)

    def score(h, qt, jp):
        # S.T chunk pair (kt = 2jp, 2jp+1) for query half qt, one head
        m, po = h // 2, 64 * (h % 2)
        ps = sps.tile([128, 1024], FP, name="ps_s", tag="s")
        for r in range(2):
            j = 2 * jp + r
            nc.tensor.matmul(
                ps[:, r * 512 : (r + 1) * 512],
                lhsT=kT_sb[po : po + 64, m, j * 128 : (j + 1) * 128],
                rhs=qT_sb[po : po + 64, m, qt * 512 : (qt + 1) * 512],
                start=True,
                stop=True,
            )
        ex = expp.tile([128, 1024], BF, name="ex", tag="ex")
        nc.scalar.activation(ex, ps, AF.Exp)
        expS[(h, qt, jp)] = ex

    def v_mt(mt):
        ps = wps.tile([128, DPC], FP, name="ps_v", tag="w")
        for c in range(KC):
            nc.tensor.matmul(
                ps,
                lhsT=xT_sb[:, c, mt * 128 : (mt + 1) * 128],
                rhs=wv_sb[:, c, :],
                start=(c == 0),
                stop=(c == KC - 1),
            )
        nc.vector.tensor_copy(v_sb[mt][:, :, 0:DH], ps)

    def ctx_pair(pair, qc):
        # ctx[q, dh|denom] for heads 2p,2p+1 in one psum tile [128, 130]
        qt = qc // 4
        pc = wps.tile([128, 130], FP, name="pc", tag="w")
        for hi in range(2):
            h = 2 * pair + hi
            col = hi * 65
            for j in range(TT):
                ex = expS[(h, qt, j // 2)]
                off = (j % 2) * 512 + (qc % 4) * 128
                nc.tensor.matmul(
                    pc[:, col : col + 65],
                    lhsT=ex[:, off : off + 128],
                    rhs=v_sb[j][:, h, :],
                    start=(j == 0),
                    stop=(j == TT - 1),
                )
        for hi in range(2):
            h = 2 * pair + hi
            k = h * QC + qc
            nc.vector.reciprocal(
                rcp_sb[:, k : k + 1], pc[:, hi * 65 + 64 : hi * 65 + 65]
            )
            nc.vector.tensor_scalar_mul(
                ctxn_sb[:, qc, h * 64 : (h + 1) * 64],
                pc[:, hi * 65 : hi * 65 + 64],
                rcp_sb[:, k : k + 1],
            )

    def tpose(pair, qc):
        tp = wps.tile([128, 128], BF, name="tp", tag="w")
        nc.tensor.matmul(
            tp,
            lhsT=ctxn_sb[:, qc, pair * 128 : (pair + 1) * 128],
            rhs=id_sb,
            is_transpose=True,
        )
        nc.vector.tensor_copy(ctxT_sb[:, pair, qc * 128 : (qc + 1) * 128], tp)

    osb_t = {}

    def out01(qc):
        osb = osbp.tile([128, D], FP, name="osb", tag="osb")
        osb_t[qc] = osb
        for n2 in range(2):
            ps = wps.tile([128, 384], FP, name="ps_o", tag="w")
            for m in range(2):
                nc.tensor.matmul(
                    ps,
                    lhsT=ctxT_sb[:, m, qc * 128 : (qc + 1) * 128],
                    rhs=wo_sb[:, m, n2 * 384 : (n2 + 1) * 384],
                    start=(m == 0),
                    stop=(m == 1),
                )
            nc.vector.tensor_copy(osb[:, n2 * 384 : (n2 + 1) * 384], ps)

    def out2(qc):
        osb = osb_t[qc]
        for n2 in range(2):
            ps = wps.tile([128, 384], FP, name="ps_o2", tag="w")
            nc.tensor.matmul(
                ps,
                lhsT=ctxT_sb[:, 2, qc * 128 : (qc + 1) * 128],
                rhs=wo_sb[:, 2, n2 * 384 : (n2 + 1) * 384],
                start=True,
                stop=True,
            )
            nc.vector.tensor_add(
                osb[:, n2 * 384 : (n2 + 1) * 384],
                osb[:, n2 * 384 : (n2 + 1) * 384],
                ps,
            )
        nc.sync.dma_start(out=out[qc * 128 : (qc + 1) * 128, :], in_=osb)

    # ---------------- schedule ----------------
    # qk m0 chase: k-n0 and q-n0 first (ScalarE copies) so head-0 scores
    # start while the n1 token-halves are still in flight.
    qk_half(0, 0, wk_sb, kT_sb, act_copy=True)
    qk_half(0, 0, wq_sb, qT_sb, act_copy=True)
    qk_half(0, 1, wk_sb, kT_sb, act_copy=False)
    score(0, 0, 0)
    score(0, 0, 1)
    qk_half(0, 1, wq_sb, qT_sb, act_copy=False)
    score(0, 0, 2)
    score(0, 0, 3)
    for jp in range(4):
        score(0, 1, jp)
    for qt in range(2):
        for jp in range(4):
            score(1, qt, jp)
    for m in (1,):
        for n in range(2):
            qk_half(m, n, wk_sb, kT_sb, act_copy=False)
            qk_half(m, n, wq_sb, qT_sb, act_copy=False)
    for mt in range(TT):
        v_mt(mt)
    for qc in range(QC):
        ctx_pair(0, qc)
    for qt in range(2):
        for jp in range(4):
            score(2, qt, jp)
    for m in (2,):
        for n in range(2):
            qk_half(m, n, wk_sb, kT_sb, act_copy=False)
            qk_half(m, n, wq_sb, qT_sb, act_copy=False)
    for qt in range(2):
        for jp in range(4):
            score(3, qt, jp)
    for qc in range(QC):
        tpose(0, qc)
    for qc in range(QC):
        ctx_pair(1, qc)
    for qt in range(2):
        for jp in range(4):
            score(4, qt, jp)
    for qc in range(QC):
        tpose(1, qc)
    for qt in range(2):
        for jp in range(4):
            score(5, qt, jp)
    for qc in range(QC):
        out01(qc)
    for qc in range(4):
        ctx_pair(2, qc)
    for qc in range(4):
        tpose(2, qc)
    for qc in range(4):
        out2(qc)
    for qc in range(4, 8):
        ctx_pair(2, qc)
    for qc in range(4, 8):
        tpose(2, qc)
    for qc in range(4, 8):
        out2(qc)


_PROGRAM = None


def build_program():
    global _PROGRAM
    if _PROGRAM is not None:
        return _PROGRAM
    nc = bacc.Bacc("TRN2", target_bir_lowering=False, debug=False, num_devices=NCORES)
    xt = nc.dram_tensor("xt", (128, KC * T), BF, kind="ExternalInput").ap()
    wk = nc.dram_tensor("wk", (128, MC * 768), BF, kind="ExternalInput").ap()
    wq = nc.dram_tensor("wq", (128, MC * 768), BF, kind="ExternalInput").ap()
    wv = nc.dram_tensor("wv", (128, KC * DPC), BF, kind="ExternalInput").ap()
    wo = nc.dram_tensor("wo", (128, MC * 768), BF, kind="ExternalInput").ap()
    ident = nc.dram_tensor("ident", (128, 128), BF, kind="ExternalInput").ap()
    out = nc.dram_tensor("out", (T, D), FP, kind="ExternalOutput").ap()
    from contextlib import ExitStack

    with TileContext(nc) as tc, ExitStack() as st:
        emit_mha(tc, xt, wk, wq, wv, wo, ident, out, st)
    nc.compile()
    _PROGRAM = nc
    return nc


def _pack_kq(w):
    # [768 d_in, 384 dout] -> [128 p, (m, c, 128)] with d_in = c*128+p
    return np.ascontiguousarray(
        w.reshape(KC, 128, MC, 128).transpose(1, 2, 0, 3).reshape(128, MC * 768)
    ).astype(BF_NP)


def make_in_maps(x, Wq, Wk, Wv, Wo):
    x = np.asarray(x, dtype=np.float32)
    ident = np.eye(128, dtype=np.float32).astype(BF_NP)
    in_maps = []
    xTs = []
    for b in range(B):
        xb = x[b].T  # [768, 1024]
        xTs.append(
            np.ascontiguousarray(
                xb.reshape(KC, 128, T).transpose(1, 0, 2).reshape(128, KC * T)
            ).astype(BF_NP)
        )
    for core in range(NCORES):
        b, hh = core // 2, core % 2
        sl = slice(hh * DPC, (hh + 1) * DPC)
        wvT = np.asarray(Wv)[sl].T.astype(np.float32)  # [768, 384]
        woT = np.asarray(Wo)[:, sl].T.astype(np.float32)  # [384, 768]
        in_maps.append(
            {
                "xt": xTs[b],
                "wq": _pack_kq((np.asarray(Wq)[sl] * 0.125).T.astype(np.float32)),
                "wk": _pack_kq(np.asarray(Wk)[sl].T.astype(np.float32)),
                "wv": np.ascontiguousarray(
                    wvT.reshape(KC, 128, DPC).transpose(1, 0, 2).reshape(128, KC * DPC)
                ).astype(BF_NP),
                "wo": np.ascontiguousarray(
                    woT.reshape(MC, 128, 768).transpose(1, 0, 2).reshape(128, MC * 768)
                ).astype(BF_NP),
                "ident": ident,
            }
        )
    return in_maps


def kernel(x, Wq, Wk, Wv, Wo, bo):
    nc = build_program()
    in_maps = make_in_maps(x, Wq, Wk, Wv, Wo)
    res = run_bass_kernel_spmd(nc, in_maps, core_ids=list(range(NCORES)))
    bo = np.asarray(bo, dtype=np.float32)
    out = np.empty((B, T, D), dtype=np.float32)
    for b in range(B):
        out[b] = res.results[2 * b]["out"] + res.results[2 * b + 1]["out"] + bo
    return out
